# revision 15
# baseline (speedup 1.0000x reference)
"""Trainium2 Bass kernel for a 4-layer hierarchical-attention encoder.

Sharding: 8 cores = 2 batch groups x 4 sequence chunks of 512 query tokens.
Each core runs the full layer stack for its 512 tokens; the hidden state is
AllGathered (feature-major, 512KB) at each layer boundary and every core
recomputes full-sequence self-attention K/V locally -- the dense projection
burst also keeps the PE at its top p-state.  Cross-attention K/V from the
static `know` are computed locally per layer during the AllGather window.

Attention is software-pipelined (scores for tile kt issue while PV matmuls
for kt-2 retire) so the PE does not sit behind the softmax exp; exp tiles
are split between the Activation engine (exact exp) and the Vector engine
(Schraudolph fp16 bit-trick exp; the ~3% per-prob error washes out in the
softmax normalization and LayerNorm).  Weights load as whole-layer slabs.
"""
import os
import sys

for _p in ("/root/.axon_site/_ro/trn_rl_repo", "/opt/trn_rl_repo", "/opt/pypackages",
           "/root/.axon_site/_ro/pypackages"):
    if os.path.isdir(_p) and _p not in sys.path:
        sys.path.append(_p)

import numpy as np

import concourse.bass as bass
import concourse.mybir as mybir
import concourse.tile as tile
from concourse import bacc
from concourse.bass_utils import run_bass_kernel_spmd

L, E, H, D, F = 4, 512, 8, 64, 2048
B, S, SK = 2, 2048, 1024
NCORES = 8
GROUPS = [[0, 1, 2, 3], [4, 5, 6, 7]]
CH = 512          # query tokens per core
ET = E // 128     # 4 feature tiles
TT = CH // 128    # 4 token tiles in own chunk
FT = F // 128     # 16 ffn tiles
KT_SA = S // 128  # 16 key tiles (self)
KT_CA = SK // 128  # 8 key tiles (cross)
HW = 65           # head width incl. denominator column
HHW = H * HW      # 520

FP32 = mybir.dt.float32
FP16 = mybir.dt.float16
I16 = mybir.dt.int16
AF = mybir.ActivationFunctionType
OP = mybir.AluOpType

# Schraudolph fp16 exp: e^(0.125*x) ~= bitcast_fp16(int16(round(A*x + B)))
SEXP_A = float(1024.0 / np.log(2.0)) * 0.125
SEXP_B = 15301.0
SEXP_MODE = os.environ.get("USE_SEXP", "0")  # 0|sa|ca|1

_CACHE = {}


def _build():
    nc = bacc.Bacc("TRN2", target_bir_lowering=False, debug=False, num_devices=NCORES)

    def din(name, shape, dt=FP16):
        return nc.dram_tensor(name, shape, dt, kind="ExternalInput").ap()

    sen_fm = din("sen_fm", [E, S])             # full batch-group seq, feature-major
    own_fm0 = din("own_fm0", [128, ET * CH])   # own chunk, feature-major packed
    own_tm0 = din("own_tm0", [CH, E])          # own chunk, token-major
    know_fm_d = din("know_fm", [E, SK])        # full know, feature-major
    ident_d = din("ident", [128, 128])
    ones_d = din("ones", [1, 128])

    wq_sa_d = din("wq_sa", [L, 128, ET * ET * 128])
    wk_sa_d = din("wk_sa", [L, 128, ET * ET * 128])
    wv_sa_d = din("wv_sa", [L, 128, ET * HHW])
    wo_sa_d = din("wo_sa", [L, 128, ET * E])
    wq_ca_d = din("wq_ca", [L, 128, ET * ET * 128])
    wk_ca_d = din("wk_ca", [L, 128, ET * ET * 128])
    wv_ca_d = din("wv_ca", [L, 128, ET * HHW])
    wo_ca_d = din("wo_ca", [L, 128, ET * E])
    w1_d = din("w1", [L, 4, 128, 4 * ET * 128])   # quarter-slabs, ft-major
    w2_d = din("w2", [L, 4, 128, 4 * E])          # quarter-slabs, ft-major

    bq_sa_d = din("bq_sa", [L, 128, ET], FP32)
    bk_sa_d = din("bk_sa", [L, 128, ET], FP32)
    bq_ca_d = din("bq_ca", [L, 128, ET], FP32)
    bk_ca_d = din("bk_ca", [L, 128, ET], FP32)
    b1_d = din("b1", [L, 128, FT], FP32)
    rbv_sa_d = din("rbv_sa", [L, 1, HHW])
    rbo_sa_d = din("rbo_sa", [L, 1, E])
    rbv_ca_d = din("rbv_ca", [L, 1, HHW])
    rbo_ca_d = din("rbo_ca", [L, 1, E])
    rb2_d = din("rb2", [L, 1, E])
    lng_d = din("lng", [L, 1, E], FP32)
    lnb_d = din("lnb", [L, 1, E], FP32)

    out_d = nc.dram_tensor("out_tm", [CH, E], FP32, kind="ExternalOutput").ap()

    with tile.TileContext(nc) as tc:
        from contextlib import ExitStack
        with ExitStack() as ctx:
            ep = ctx.enter_context
            const_p = ep(tc.tile_pool(name="const", bufs=1))
            know_p = ep(tc.tile_pool(name="know", bufs=4))
            hch_p = ep(tc.tile_pool(name="hch", bufs=6))      # [128,512] hidden fm
            kfm_p = ep(tc.tile_pool(name="kfm", bufs=4))      # [128,2048] SA K fp16
            vsa_p = ep(tc.tile_pool(name="vsa", bufs=4))      # [128,2080] SA V fp16
            kca_p = ep(tc.tile_pool(name="kca", bufs=4))      # [128,1024] CA K fp16
            vca_p = ep(tc.tile_pool(name="vca", bufs=4))      # [128,1040] CA V fp16
            xfm_p = ep(tc.tile_pool(name="xfm", bufs=4))      # [128,2048] fm acts
            qfm_p = ep(tc.tile_pool(name="qfm", bufs=4))
            attn_p = ep(tc.tile_pool(name="attn", bufs=7))
            stm_p = ep(tc.tile_pool(name="stm", bufs=9))     # hid/inter/co TM fp16
            out32_p = ep(tc.tile_pool(name="out32", bufs=4))
            pt_p = ep(tc.tile_pool(name="pt", bufs=4))        # exp(scores) rings
            gel_p = ep(tc.tile_pool(name="gel", bufs=3))
            wsl_p = ep(tc.tile_pool(name="wsl", bufs=1))      # weight slabs
            row_p = ep(tc.tile_pool(name="row", bufs=4))      # [1,<=520] rows
            gb_p = ep(tc.tile_pool(name="gb", bufs=1))        # LN G/B broadcast fp32
            sc_p = ep(tc.tile_pool(name="sc", bufs=2))        # fp32 scratch
            s1_p = ep(tc.tile_pool(name="s1", bufs=1))        # [<=4,512] rows
            st_p = ep(tc.tile_pool(name="st", bufs=8))        # small stats
            ps_p = ep(tc.tile_pool(name="ps", bufs=6, space="PSUM"))
            dram_p = ep(tc.tile_pool(name="dram", bufs=2, space="DRAM"))

            identt = const_p.tile([128, 128], FP16, tag="ident", name="ident")
            nc.sync.dma_start(identt[:], ident_d[:])
            onest = const_p.tile([1, 128], FP16, tag="ones", name="ones")
            nc.sync.dma_start(onest[:], ones_d[:])
            knowfm = []
            for e in range(ET):
                t = know_p.tile([128, SK], FP16, tag="know", name="know")
                nc.sync.dma_start(t[:], know_fm_d[e * 128:(e + 1) * 128, :])
                knowfm.append(t)
            ownfm = xfm_p.tile([128, ET * CH], FP16, tag="xfm", name="ownfm")
            nc.sync.dma_start(ownfm[:], own_fm0[:])
            hid = []
            for t in range(TT):
                h = stm_p.tile([128, E], FP16, tag="stm", name="hid")
                nc.sync.dma_start(h[:], own_tm0[t * 128:(t + 1) * 128, :])
                hid.append(h)

            # ---------------- weight loaders (single-buffered slabs) ----
            def slab(dram, l, cols, tg, bufs=1):
                t = wsl_p.tile([128, cols], FP16, tag=tg, name=tg, bufs=bufs)
                nc.sync.dma_start(t[:], dram[l])
                return t

            def srow(dram, l, cols, tg="row"):
                t = row_p.tile([1, cols], FP16, tag=tg, name=tg, bufs=8)
                nc.sync.dma_start(t[:], dram[l])
                return t

            def sbias(dram, l, cols, tg="bias"):
                t = st_p.tile([128, cols], FP32, tag=tg, name=tg, bufs=8)
                nc.sync.dma_start(t[:], dram[l])
                return t

            def load_sa_kvq(l):
                return dict(
                    wq=slab(wq_sa_d, l, ET * ET * 128, "wq_sa"),
                    wk=slab(wk_sa_d, l, ET * ET * 128, "wk_sa"),
                    wv=slab(wv_sa_d, l, ET * HHW, "wv_sa"),
                    bq=sbias(bq_sa_d, l, ET), bk=sbias(bk_sa_d, l, ET),
                    rbv=srow(rbv_sa_d, l, HHW))

            def load_sa_o(l):
                return dict(wo=slab(wo_sa_d, l, ET * E, "wo_sa"),
                            rbo=srow(rbo_sa_d, l, E))

            def load_ca_kv(l):
                return dict(
                    wk=slab(wk_ca_d, l, ET * ET * 128, "wk_ca"),
                    wv=slab(wv_ca_d, l, ET * HHW, "wv_ca"),
                    bk=sbias(bk_ca_d, l, ET), rbv=srow(rbv_ca_d, l, HHW))

            def load_ca_qo(l):
                return dict(
                    wq=slab(wq_ca_d, l, ET * ET * 128, "wq_ca"),
                    wo=slab(wo_ca_d, l, ET * E, "wo_ca"),
                    bq=sbias(bq_ca_d, l, ET), rbo=srow(rbo_ca_d, l, E))

            def load_ffn_w(l):
                return dict(
                    w1=[slab(w1_d[l], q, 4 * ET * 128, "w1q", bufs=2)
                        for q in range(4)],
                    w2=[slab(w2_d[l], q, 4 * E, "w2q", bufs=2) for q in range(4)],
                    b1=sbias(b1_d, l, FT), rb2=srow(rb2_d, l, E))

            def load_ln(l):
                lr = s1_p.tile([1, E], FP32, tag="lnrow", name="lnrow")
                nc.sync.dma_start(lr[:], lng_d[l])
                G = gb_p.tile([128, E], FP32, tag="G", name="G")
                nc.gpsimd.partition_broadcast(G[:], lr[:])
                lr2 = s1_p.tile([1, E], FP32, tag="B", name="lnrow2")
                nc.sync.dma_start(lr2[:], lnb_d[l])
                Bt = gb_p.tile([128, E], FP32, tag="Bb", name="Bb")
                nc.gpsimd.partition_broadcast(Bt[:], lr2[:])
                return G, Bt

            # ---------------- compute helpers ----------------
            I32 = mybir.dt.int32

            def rsqrt4(mv8):
                """inv4 = rsqrt(var*E/(E-1)), minv4 = mean*inv4; vars at odd cols."""
                v4 = st_p.tile([128, 4], FP32, tag="v4", name="v4")
                nc.vector.tensor_scalar_mul(v4[:], mv8[:, 1:8:2], float(E) / (E - 1))
                h4 = st_p.tile([128, 4], FP32, tag="h4", name="h4")
                nc.vector.tensor_scalar_mul(h4[:], v4[:], 0.5)
                t1 = st_p.tile([128, 4], I32, tag="t1", name="t1")
                nc.vector.tensor_scalar(t1[:], in0=v4[:].bitcast(I32), scalar1=1,
                                        scalar2=None, op0=OP.arith_shift_right)
                y4 = st_p.tile([128, 4], FP32, tag="y4", name="y4")
                nc.vector.tensor_scalar(y4[:].bitcast(I32), in0=t1[:], scalar1=-1,
                                        scalar2=0x5f3759df, op0=OP.mult, op1=OP.add)
                for _ in range(2):
                    sq = st_p.tile([128, 4], FP32, tag="sq", name="sq")
                    nc.vector.tensor_mul(sq[:], y4[:], y4[:])
                    nc.vector.tensor_mul(sq[:], sq[:], h4[:])
                    nc.vector.tensor_scalar(sq[:], in0=sq[:], scalar1=-1.0,
                                            scalar2=1.5, op0=OP.mult, op1=OP.add)
                    nc.vector.tensor_mul(y4[:], y4[:], sq[:])
                m4 = st_p.tile([128, 4], FP32, tag="m4", name="m4")
                nc.vector.tensor_mul(m4[:], mv8[:, 0:8:2], y4[:])
                return y4, m4

            def ln_norm(xres, G, Bt, out):
                """out = G*(xres-mean)/(sqrt(bessel_var)+eps) + Bt."""
                stt = st_p.tile([128, 6], FP32, tag="bnst", name="bnst")
                nc.vector.bn_stats(out=stt[:], in_=xres[:])
                mv = st_p.tile([128, 2], FP32, tag="bnmv", name="bnmv")
                nc.vector.bn_aggr(out=mv[:], in_=stt[:])
                sd = st_p.tile([128, 1], FP32, tag="sd", name="sd")
                nc.scalar.activation(sd[:], mv[:, 1:2], AF.Sqrt, scale=float(E) / (E - 1))
                nc.vector.tensor_scalar_add(sd[:], sd[:], 1e-6)
                inv = st_p.tile([128, 1], FP32, tag="inv", name="inv")
                nc.vector.reciprocal_approx_fast(inv[:], sd[:])
                minv = st_p.tile([128, 1], FP32, tag="minv", name="minv")
                nc.vector.tensor_mul(minv[:], mv[:, 0:1], inv[:])
                tmp = sc_p.tile([128, E], FP32, tag="lntmp", name="lntmp")
                nc.vector.tensor_scalar(tmp[:], in0=xres[:], scalar1=inv[:],
                                        scalar2=minv[:], op0=OP.mult, op1=OP.subtract)
                nc.vector.tensor_mul(tmp[:], tmp[:], G[:])
                nc.vector.tensor_add(out[:], tmp[:], Bt[:])

            def transpose_to(dst, src_tile, t):
                """src [128tok, E] TM tile t -> dst[:, e*CH + t*128 ...] (fm)."""
                for e in range(ET):
                    tp = ps_p.tile([128, 128], FP16, tag="ps", name="ps")
                    nc.tensor.transpose(tp[:], src_tile[:, e * 128:(e + 1) * 128],
                                        identt[:])
                    nc.vector.tensor_copy(dst[:, e * CH + t * 128:e * CH + (t + 1) * 128],
                                          tp[:])

            def q_proj(src_fm, wq, bq):
                qs = []
                for e in range(ET):
                    pst = ps_p.tile([128, CH], FP32, tag="ps", name="ps")
                    for ei in range(ET):
                        nc.tensor.matmul(
                            pst[:], wq[:, (ei * ET + e) * 128:(ei * ET + e + 1) * 128],
                            src_fm[:, ei * CH:(ei + 1) * CH],
                            start=(ei == 0), stop=(ei == ET - 1))
                    qt = qfm_p.tile([128, CH], FP16, tag="qfm", name="qfm")
                    nc.vector.tensor_scalar_add(qt[:], pst[:], bq[:, e:e + 1])
                    qs.append(qt)
                return qs

            def kv_all(w, src_fn):
                """Full-sequence SA K/V from per-chunk hidden fm tiles.

                src_fn(ch) -> list of 4 [128,512] fm tiles for chunk ch."""
                kfm = [kfm_p.tile([128, S], FP16, tag="kfm", name="kfm")
                       for _ in range(ET)]
                vsa = [vsa_p.tile([128, 4 * HHW], FP16, tag="vsa", name="vsa")
                       for _ in range(4)]
                for ch in range(4):
                    hch = src_fn(ch)
                    for e in range(ET):
                        pst = ps_p.tile([128, CH], FP32, tag="ps", name="ps")
                        for ei in range(ET):
                            nc.tensor.matmul(
                                pst[:],
                                w["wk"][:, (ei * ET + e) * 128:(ei * ET + e + 1) * 128],
                                hch[ei][:], start=(ei == 0), stop=(ei == ET - 1))
                        nc.scalar.activation(
                            kfm[e][:, ch * CH:(ch + 1) * CH], pst[:], AF.Identity,
                            bias=w["bk"][:, e:e + 1])
                    for lt in range(TT):
                        for half in range(2):
                            cs = half * (HHW // 2)
                            pst = ps_p.tile([128, HHW // 2], FP32, tag="ps", name="ps")
                            for ei in range(ET):
                                nc.tensor.matmul(
                                    pst[:], hch[ei][:, lt * 128:(lt + 1) * 128],
                                    w["wv"][:, ei * HHW + cs:ei * HHW + cs + HHW // 2],
                                    start=(ei == 0), stop=False)
                            nc.tensor.matmul(pst[:], onest[:],
                                             w["rbv"][:, cs:cs + HHW // 2],
                                             start=False, stop=True)
                            nc.scalar.activation(
                                vsa[ch][:, lt * HHW + cs:lt * HHW + cs + HHW // 2],
                                pst[:], AF.Copy)
                return kfm, vsa

            def make_ca_kv(w):
                """Full CA K/V from resident know (collective-window filler)."""
                kca = [kca_p.tile([128, SK], FP16, tag="kca", name="kca")
                       for _ in range(ET)]
                vca = [vca_p.tile([128, 2 * HHW], FP16, tag="vca", name="vca")
                       for _ in range(4)]
                for e in range(ET):
                    for cc in range(2):
                        pst = ps_p.tile([128, CH], FP32, tag="ps", name="ps")
                        for ei in range(ET):
                            nc.tensor.matmul(
                                pst[:],
                                w["wk"][:, (ei * ET + e) * 128:(ei * ET + e + 1) * 128],
                                knowfm[ei][:, cc * CH:(cc + 1) * CH],
                                start=(ei == 0), stop=(ei == ET - 1))
                        nc.scalar.activation(
                            kca[e][:, cc * CH:(cc + 1) * CH], pst[:], AF.Identity,
                            bias=w["bk"][:, e:e + 1])
                for kt in range(KT_CA):
                    for half in range(2):
                        cs = half * (HHW // 2)
                        pst = ps_p.tile([128, HHW // 2], FP32, tag="ps", name="ps")
                        for ei in range(ET):
                            nc.tensor.matmul(
                                pst[:], knowfm[ei][:, kt * 128:(kt + 1) * 128],
                                w["wv"][:, ei * HHW + cs:ei * HHW + cs + HHW // 2],
                                start=(ei == 0), stop=False)
                        nc.tensor.matmul(pst[:], onest[:], w["rbv"][:, cs:cs + HHW // 2],
                                         start=False, stop=True)
                        nc.scalar.activation(
                            vca[kt // 2][:, (kt % 2) * HHW + cs:(kt % 2) * HHW + cs + HHW // 2],
                            pst[:], AF.Copy)
                return kca, vca

            def attention(qfm, kfm, vp_at, nkt, attn_tiles, sexp=False):
                PD = 2  # exp pipeline distance in kt tiles
                for hp in range(ET):
                    attps = [ps_p.tile([HW, CH], FP32, tag="attps", name="attps",
                                       bufs=2)
                             for _ in range(2)]
                    pts = {}

                    def scores(kt):
                        for j in (0, 1):
                            spt = ps_p.tile([128, CH], FP32, tag="ps", name="spt")
                            nc.tensor.matmul(
                                spt[:],
                                kfm[hp][j * 64:(j + 1) * 64, kt * 128:(kt + 1) * 128],
                                qfm[hp][j * 64:(j + 1) * 64, :], start=True, stop=True)
                            if j == 0 or not sexp:
                                pt = pt_p.tile([128, CH], FP16, tag="pte", name="pte")
                                nc.scalar.activation(pt[:], spt[:], AF.Exp, scale=0.125)
                                pts[kt, j] = pt[:]
                            else:
                                pti = pt_p.tile([128, CH], FP16, tag="ptv", name="ptv")
                                nc.vector.tensor_scalar(pti[:].bitcast(I16), in0=spt[:],
                                                        scalar1=SEXP_A, scalar2=SEXP_B,
                                                        op0=OP.mult, op1=OP.add)
                                pts[kt, j] = pti[:]

                    def pv(kt):
                        for j in (0, 1):
                            h = hp * 2 + j
                            vtile, col0 = vp_at(kt, h)
                            nc.tensor.matmul(attps[j][:], vtile[:, col0:col0 + HW],
                                             pts.pop((kt, j)),
                                             start=(kt == 0), stop=(kt == nkt - 1))

                    for kt in range(nkt):
                        scores(kt)
                        if kt >= PD:
                            pv(kt - PD)
                    for kt in range(nkt - PD, nkt):
                        pv(kt)
                    for j in (0, 1):
                        den = s1_p.tile([1, CH], FP32, tag="den", name="den")
                        nc.scalar.activation(den[:], attps[j][64:65, :], AF.Copy)
                        rec = s1_p.tile([1, CH], FP32, tag="rec", name="rec")
                        nc.vector.reciprocal_approx_fast(rec[:], den[:])
                        rb = sc_p.tile([64, CH], FP32, tag="rb", name="rb")
                        nc.gpsimd.partition_broadcast(rb[:], rec[:])
                        nc.vector.tensor_mul(attn_tiles[hp][j * 64:(j + 1) * 64, :],
                                             attps[j][0:64, :], rb[:])

            def ln_finish(xs, mv8, G, Bt, out_tiles):
                inv4, minv4 = rsqrt4(mv8)
                for t in range(TT):
                    tmp = sc_p.tile([128, E], FP32, tag="lntmp", name="lntmp")
                    nc.vector.tensor_scalar(tmp[:], in0=xs[t][:],
                                            scalar1=inv4[:, t:t + 1],
                                            scalar2=minv4[:, t:t + 1],
                                            op0=OP.mult, op1=OP.subtract)
                    nc.vector.tensor_mul(tmp[:], tmp[:], G[:])
                    nc.vector.tensor_add(out_tiles[t][:], tmp[:], Bt[:])

            def out_proj_ln(attn_tiles, w, res, G, Bt, out_tiles):
                mv8 = st_p.tile([128, 8], FP32, tag="mv8", name="mv8", bufs=4)
                xs = []
                for t in range(TT):
                    pst = ps_p.tile([128, E], FP32, tag="ps", name="ps")
                    for ei in range(ET):
                        nc.tensor.matmul(pst[:], attn_tiles[ei][:, t * 128:(t + 1) * 128],
                                         w["wo"][:, ei * E:(ei + 1) * E],
                                         start=(ei == 0), stop=False)
                    nc.tensor.matmul(pst[:], onest[:], w["rbo"][:], start=False, stop=True)
                    xres = sc_p.tile([128, E], FP32, tag="xres", name="xres", bufs=4)
                    nc.vector.tensor_add(xres[:], pst[:], res[t][:])
                    stt = st_p.tile([128, 6], FP32, tag="bnst", name="bnst")
                    nc.vector.bn_stats(out=stt[:], in_=xres[:])
                    nc.vector.bn_aggr(out=mv8[:, 2 * t:2 * t + 2], in_=stt[:])
                    xs.append(xres)
                ln_finish(xs, mv8, G, Bt, out_tiles)

            def hch_from_sen(ch):
                tiles = []
                for ei in range(ET):
                    t = hch_p.tile([128, CH], FP16, tag="hch", name="hch")
                    nc.sync.dma_start(t[:], sen_fm[ei * 128:(ei + 1) * 128,
                                                   ch * CH:(ch + 1) * CH])
                    tiles.append(t)
                return tiles

            def hch_from_ag(ag_out, ch):
                tiles = []
                for ei in range(ET):
                    t = hch_p.tile([128, CH], FP16, tag="hch", name="hch")
                    nc.sync.dma_start(
                        t[:], ag_out[ch * E + ei * 128:ch * E + (ei + 1) * 128, :])
                    tiles.append(t)
                return tiles

            # ---------------- bootstrap: layer-0 K/V + CA K/V ----------------
            sa_kvq = load_sa_kvq(0)
            ca_kv = load_ca_kv(0)
            G, Bt = load_ln(0)
            kfm, vsa = kv_all(sa_kvq, hch_from_sen)
            qsa = q_proj(ownfm, sa_kvq["wq"], sa_kvq["bq"])
            kca, vca = make_ca_kv(ca_kv)
            ag_out_cur = None

            for l in range(L):
                with nc.named_scope(f"L{l}"):
                    if l > 0:
                        kfm, vsa = kv_all(
                            sa_kvq, lambda ch: hch_from_ag(ag_out_cur, ch))
                    sa_o = load_sa_o(l)
                    ca_qo = load_ca_qo(l)
                    ffn_w = load_ffn_w(l)
                    if l < L - 1:
                        ca_kv_next = load_ca_kv(l + 1)

                    # ---- SA attention ----
                    attn = [attn_p.tile([128, CH], FP16, tag="attn", name="attn")
                            for _ in range(ET)]
                    with nc.named_scope("sa"):
                        attention(qsa, kfm,
                                  lambda kt, h: (vsa[kt // 4], (kt % 4) * HHW + h * HW),
                                  KT_SA, attn, sexp=SEXP_MODE in ("1", "sa"))

                    inter = [stm_p.tile([128, E], FP16, tag="stm", name="inter")
                             for _ in range(TT)]
                    with nc.named_scope("oln1"):
                        out_proj_ln(attn, sa_o, hid, G, Bt, inter)
                        interfm = xfm_p.tile([128, ET * CH], FP16, tag="xfm",
                                             name="interfm")
                        for t in range(TT):
                            transpose_to(interfm, inter[t], t)

                    # ---- CA ----
                    with nc.named_scope("ca"):
                        qca = q_proj(interfm, ca_qo["wq"], ca_qo["bq"])
                        if l < L - 1:
                            sa_kvq_next = load_sa_kvq(l + 1)
                        attn2 = [attn_p.tile([128, CH], FP16, tag="attn", name="attn2")
                                 for _ in range(ET)]
                        attention(qca, kca,
                                  lambda kt, h: (vca[kt // 2], (kt % 2) * HHW + h * HW),
                                  KT_CA, attn2, sexp=SEXP_MODE in ("1", "ca"))
                    co = [stm_p.tile([128, E], FP16, tag="stm", name="co")
                          for _ in range(TT)]
                    with nc.named_scope("oln2"):
                        out_proj_ln(attn2, ca_qo, inter, G, Bt, co)
                        if l < L - 1:
                            kca, vca = make_ca_kv(ca_kv_next)
                        cofm = xfm_p.tile([128, ET * CH], FP16, tag="xfm", name="cofm")
                        for t in range(TT):
                            transpose_to(cofm, co[t], t)

                    # ---- FFN (h1/gelu/h2 interleaved, distance 2) ----
                    with nc.named_scope("ffn"):
                        if l == L - 1:
                            hidn = [out32_p.tile([128, E], FP32, tag="out32",
                                                 name="out32") for _ in range(TT)]
                        else:
                            hidn = [stm_p.tile([128, E], FP16, tag="stm", name="hidn")
                                    for _ in range(TT)]
                        h2ps = [ps_p.tile([128, E], FP32, tag="ps", name="ps")
                                for _ in range(TT)]
                        gel = {}

                        def h2_emit(ft):
                            gt = gel.pop(ft)
                            for t in range(TT):
                                nc.tensor.matmul(h2ps[t][:], gt[:, t * 128:(t + 1) * 128],
                                                 ffn_w["w2"][ft // 4][:,
                                                 (ft % 4) * E:(ft % 4 + 1) * E],
                                                 start=(ft == 0), stop=False)

                        for ft in range(FT):
                            pst = ps_p.tile([128, CH], FP32, tag="ps", name="ps")
                            w1q = ffn_w["w1"][ft // 4]
                            for ei in range(ET):
                                nc.tensor.matmul(
                                    pst[:],
                                    w1q[:, ((ft % 4) * ET + ei) * 128:
                                        ((ft % 4) * ET + ei + 1) * 128],
                                    cofm[:, ei * CH:(ei + 1) * CH],
                                    start=(ei == 0), stop=(ei == ET - 1))
                            gt = gel_p.tile([128, CH], FP16, tag="gel", name="gel")
                            nc.scalar.activation(gt[:], pst[:], AF.Gelu,
                                                 bias=ffn_w["b1"][:, ft:ft + 1])
                            gel[ft] = gt
                            if ft >= 2:
                                h2_emit(ft - 2)
                        h2_emit(FT - 2)
                        h2_emit(FT - 1)
                        mv8 = st_p.tile([128, 8], FP32, tag="mv8", name="mv8",
                                        bufs=4)
                        xs = []
                        for t in range(TT):
                            nc.tensor.matmul(h2ps[t][:], onest[:], ffn_w["rb2"][:],
                                             start=False, stop=True)
                            xres = sc_p.tile([128, E], FP32, tag="xres", name="xres",
                                             bufs=4)
                            nc.vector.tensor_add(xres[:], h2ps[t][:], co[t][:])
                            stt = st_p.tile([128, 6], FP32, tag="bnst", name="bnst")
                            nc.vector.bn_stats(out=stt[:], in_=xres[:])
                            nc.vector.bn_aggr(out=mv8[:, 2 * t:2 * t + 2], in_=stt[:])
                            xs.append(xres)
                        ln_finish(xs, mv8, G, Bt, hidn)
                        if l == L - 1:
                            for t in range(TT):
                                nc.sync.dma_start(out_d[t * 128:(t + 1) * 128, :],
                                                  hidn[t][:])

                    # ---- boundary: AllGather hidden; CA K/V + next Q fill it ----
                    if l < L - 1:
                        with nc.named_scope("bnd"):
                            ownfm_n = xfm_p.tile([128, ET * CH], FP16, tag="xfm",
                                                 name="ownfm_n")
                            for t in range(TT):
                                transpose_to(ownfm_n, hidn[t], t)
                            ag_in = dram_p.tile([E, CH], FP16, tag="agin", name="agin")
                            for e in range(ET):
                                nc.scalar.dma_start(
                                    ag_in[e * 128:(e + 1) * 128, :],
                                    ownfm_n[:, e * CH:(e + 1) * CH])
                            ag_out_cur = dram_p.tile([4 * E, CH], FP16, tag="agout",
                                                     name="agout")
                            nc.gpsimd.collective_compute(
                                "AllGather", OP.bypass, replica_groups=GROUPS,
                                ins=[ag_in.opt()], outs=[ag_out_cur.opt()])
                            qsa = q_proj(ownfm_n, sa_kvq_next["wq"], sa_kvq_next["bq"])
                            Gn, Btn = load_ln(l + 1)
                        sa_kvq, ca_kv, G, Bt = sa_kvq_next, ca_kv_next, Gn, Btn
                        hid = hidn

    nc.compile()
    return nc


def _pack_ee(w):
    """[L,E,E] -> [L,128, ET*ET*128] slab: cols (ei,e,c), lhsT tile (ei,e)."""
    return np.ascontiguousarray(
        w.reshape(L, ET, 128, ET, 128).transpose(0, 2, 1, 3, 4)
        .reshape(L, 128, ET * ET * 128).astype(np.float16))


def _fm_pack(x_fm):
    """[E, T] -> [128, ET*T] (cols (e,t))."""
    t = x_fm.shape[1]
    return np.ascontiguousarray(
        x_fm.reshape(ET, 128, t).transpose(1, 0, 2).reshape(128, ET * t)
        .astype(np.float16))


def _prep_inputs(sen, know, sa_qkv_w, sa_qkv_b, sa_out_w, sa_out_b,
                 ca_qkv_w, ca_qkv_b, ca_out_w, ca_out_b,
                 ff_w1, ff_b1, ff_w2, ff_b2, ln_g, ln_b):
    f16, f32 = np.float16, np.float32

    def padv(w, b):  # [L,E,E],[L,E] -> [L,128,ET*HHW], [L,1,HHW]
        wp = np.zeros((L, E, H, HW), f32)
        wp[:, :, :, :D] = w.reshape(L, E, H, D)
        bp = np.zeros((L, H, HW), f32)
        bp[:, :, :D] = b.reshape(L, H, D)
        bp[:, :, D] = 1.0
        wsl = wp.reshape(L, ET, 128, H * HW).transpose(0, 2, 1, 3).reshape(
            L, 128, ET * HHW)
        return (np.ascontiguousarray(wsl.astype(f16)),
                np.ascontiguousarray(bp.reshape(L, 1, HHW).astype(f16)))

    wv_sa_p, rbv_sa_h = padv(sa_qkv_w[:, 2], sa_qkv_b[:, 2])
    wv_ca_p, rbv_ca_h = padv(ca_qkv_w[:, 2], ca_qkv_b[:, 2])

    def pack_o(w):  # [L,E,E] -> [L,128,ET*E]
        return np.ascontiguousarray(
            w.reshape(L, ET, 128, E).transpose(0, 2, 1, 3).reshape(L, 128, ET * E)
            .astype(f16))

    # w1: [L,E,F] -> quarter-slabs [L,4,128,4*ET*128], cols (ft%4, ei, c)
    w1q = (ff_w1.reshape(L, ET, 128, 4, 4, 128)      # (ei,p,q,ftq,c)
           .transpose(0, 3, 2, 4, 1, 5)              # (L,q,p,ftq,ei,c)
           .reshape(L, 4, 128, 4 * ET * 128))
    # w2: [L,F,E] -> quarter-slabs [L,4,128,4*E], cols (ft%4, c)
    w2q = (ff_w2.reshape(L, 4, 4, 128, E)            # (q,ftq,p,c)
           .transpose(0, 1, 3, 2, 4)                 # (L,q,p,ftq,c)
           .reshape(L, 4, 128, 4 * E))

    common = {
        "ident": np.eye(128, dtype=f16),
        "ones": np.ones((1, 128), f16),
        "wq_sa": _pack_ee(sa_qkv_w[:, 0]), "wk_sa": _pack_ee(sa_qkv_w[:, 1]),
        "wv_sa": wv_sa_p, "wo_sa": pack_o(sa_out_w),
        "wq_ca": _pack_ee(ca_qkv_w[:, 0]), "wk_ca": _pack_ee(ca_qkv_w[:, 1]),
        "wv_ca": wv_ca_p, "wo_ca": pack_o(ca_out_w),
        "w1": np.ascontiguousarray(w1q.astype(f16)),
        "w2": np.ascontiguousarray(w2q.astype(f16)),
        "bq_sa": np.ascontiguousarray(
            sa_qkv_b[:, 0].reshape(L, ET, 128).transpose(0, 2, 1)),
        "bk_sa": np.ascontiguousarray(
            sa_qkv_b[:, 1].reshape(L, ET, 128).transpose(0, 2, 1)),
        "bq_ca": np.ascontiguousarray(
            ca_qkv_b[:, 0].reshape(L, ET, 128).transpose(0, 2, 1)),
        "bk_ca": np.ascontiguousarray(
            ca_qkv_b[:, 1].reshape(L, ET, 128).transpose(0, 2, 1)),
        "b1": np.ascontiguousarray(ff_b1.reshape(L, FT, 128).transpose(0, 2, 1)),
        "rbv_sa": rbv_sa_h, "rbv_ca": rbv_ca_h,
        "rbo_sa": np.ascontiguousarray(sa_out_b[:, None, :].astype(f16)),
        "rbo_ca": np.ascontiguousarray(ca_out_b[:, None, :].astype(f16)),
        "rb2": np.ascontiguousarray(ff_b2[:, None, :].astype(f16)),
        "lng": np.ascontiguousarray(ln_g[:, None, :]),
        "lnb": np.ascontiguousarray(ln_b[:, None, :]),
    }
    in_maps = []
    for core in range(NCORES):
        g, c = core // 4, core % 4
        m = dict(common)
        m["sen_fm"] = np.ascontiguousarray(sen[g].T.astype(f16))
        m["own_fm0"] = _fm_pack(sen[g, c * CH:(c + 1) * CH].T)
        m["own_tm0"] = np.ascontiguousarray(sen[g, c * CH:(c + 1) * CH].astype(f16))
        m["know_fm"] = np.ascontiguousarray(know[g].T.astype(f16))
        in_maps.append(m)
    return in_maps


def kernel(**inputs):
    inputs = {k: np.asarray(v, dtype=np.float32) for k, v in inputs.items()}
    if "nc" not in _CACHE:
        _CACHE["nc"] = _build()
    nc = _CACHE["nc"]
    in_maps = _prep_inputs(**inputs)
    res = run_bass_kernel_spmd(nc, in_maps, list(range(NCORES)))
    out = np.empty((B, S, E), np.float32)
    for core in range(NCORES):
        g, c = core // 4, core % 4
        out[g, c * CH:(c + 1) * CH] = res.results[core]["out_tm"]
    return out


# revision 16
# speedup vs baseline: 1.0506x; 1.0506x over previous
"""Trainium2 Bass kernel for a 4-layer hierarchical-attention encoder.

Sharding: 8 cores = 2 batch groups x 4 sequence chunks of 512 query tokens.
Each core runs the full layer stack for its 512 tokens; the hidden state is
AllGathered (feature-major, 512KB) at each layer boundary and every core
recomputes full-sequence self-attention K/V locally -- the dense projection
burst also keeps the PE at its top p-state.  Cross-attention K/V from the
static `know` are computed locally per layer during the AllGather window.

Attention is software-pipelined (scores for tile kt issue while PV matmuls
for kt-2 retire) so the PE does not sit behind the softmax exp; exp tiles
are split between the Activation engine (exact exp) and the Vector engine
(Schraudolph fp16 bit-trick exp; the ~3% per-prob error washes out in the
softmax normalization and LayerNorm).  Weights load as whole-layer slabs.
"""
import os
import sys

for _p in ("/root/.axon_site/_ro/trn_rl_repo", "/opt/trn_rl_repo", "/opt/pypackages",
           "/root/.axon_site/_ro/pypackages"):
    if os.path.isdir(_p) and _p not in sys.path:
        sys.path.append(_p)

import numpy as np

import concourse.bass as bass
import concourse.mybir as mybir
import concourse.tile as tile
from concourse import bacc
from concourse.bass_utils import run_bass_kernel_spmd
from concourse.bass import ts

L, E, H, D, F = 4, 512, 8, 64, 2048
B, S, SK = 2, 2048, 1024
NCORES = 8
GROUPS = [[0, 1, 2, 3], [4, 5, 6, 7]]
CH = 512          # query tokens per core
ET = E // 128     # 4 feature tiles
TT = CH // 128    # 4 token tiles in own chunk
FT = F // 128     # 16 ffn tiles
KT_SA = S // 128  # 16 key tiles (self)
KT_CA = SK // 128  # 8 key tiles (cross)
HW = 65           # head width incl. denominator column
HHW = H * HW      # 520

FP32 = mybir.dt.float32
FP16 = mybir.dt.float16
I16 = mybir.dt.int16
AF = mybir.ActivationFunctionType
OP = mybir.AluOpType

# Schraudolph fp16 exp: e^(0.125*x) ~= bitcast_fp16(int16(round(A*x + B)))
SEXP_A = float(1024.0 / np.log(2.0)) * 0.125
SEXP_B = 15301.0
SEXP_MODE = os.environ.get("USE_SEXP", "0")  # 0|sa|ca|1

_CACHE = {}


def _build():
    nc = bacc.Bacc("TRN2", target_bir_lowering=False, debug=False, num_devices=NCORES)

    def din(name, shape, dt=FP16):
        return nc.dram_tensor(name, shape, dt, kind="ExternalInput").ap()

    sen_fm = din("sen_fm", [E, S])             # full batch-group seq, feature-major
    own_fm0 = din("own_fm0", [128, ET * CH])   # own chunk, feature-major packed
    own_tm0 = din("own_tm0", [CH, E])          # own chunk, token-major
    know_fm_d = din("know_fm", [E, SK])        # full know, feature-major
    ident_d = din("ident", [128, 128])
    ones_d = din("ones", [1, 128])

    wq_sa_d = din("wq_sa", [L, 128, ET * ET * 128])
    wk_sa_d = din("wk_sa", [L, 128, ET * ET * 128])
    wv_sa_d = din("wv_sa", [L, 128, ET * HHW])
    wo_sa_d = din("wo_sa", [L, 128, ET * E])
    wq_ca_d = din("wq_ca", [L, 128, ET * ET * 128])
    wk_ca_d = din("wk_ca", [L, 128, ET * ET * 128])
    wv_ca_d = din("wv_ca", [L, 128, ET * HHW])
    wo_ca_d = din("wo_ca", [L, 128, ET * E])
    w1_d = din("w1", [L, 4, 128, 4 * ET * 128])   # quarter-slabs, ft-major
    w2_d = din("w2", [L, 4, 128, 4 * E])          # quarter-slabs, ft-major

    bq_sa_d = din("bq_sa", [L, 128, ET], FP32)
    bk_sa_d = din("bk_sa", [L, 128, ET], FP32)
    bq_ca_d = din("bq_ca", [L, 128, ET], FP32)
    bk_ca_d = din("bk_ca", [L, 128, ET], FP32)
    b1_d = din("b1", [L, 128, FT], FP32)
    rbv_sa_d = din("rbv_sa", [L, 1, HHW])
    rbo_sa_d = din("rbo_sa", [L, 1, E])
    rbv_ca_d = din("rbv_ca", [L, 1, HHW])
    rbo_ca_d = din("rbo_ca", [L, 1, E])
    rb2_d = din("rb2", [L, 1, E])
    lng_d = din("lng", [L, 1, E], FP32)
    lnb_d = din("lnb", [L, 1, E], FP32)

    out_d = nc.dram_tensor("out_tm", [CH, E], FP32, kind="ExternalOutput").ap()

    with tile.TileContext(nc) as tc:
        from contextlib import ExitStack
        with ExitStack() as ctx:
            ep = ctx.enter_context
            const_p = ep(tc.tile_pool(name="const", bufs=1))
            know_p = ep(tc.tile_pool(name="know", bufs=4))
            hch_p = ep(tc.tile_pool(name="hch", bufs=6))      # [128,512] hidden fm
            kfm_p = ep(tc.tile_pool(name="kfm", bufs=4))      # [128,2048] SA K fp16
            vsa_p = ep(tc.tile_pool(name="vsa", bufs=4))      # [128,2080] SA V fp16
            kca_p = ep(tc.tile_pool(name="kca", bufs=4))      # [128,1024] CA K fp16
            vca_p = ep(tc.tile_pool(name="vca", bufs=4))      # [128,1040] CA V fp16
            xfm_p = ep(tc.tile_pool(name="xfm", bufs=4))      # [128,2048] fm acts
            qfm_p = ep(tc.tile_pool(name="qfm", bufs=4))
            attn_p = ep(tc.tile_pool(name="attn", bufs=7))
            stm_p = ep(tc.tile_pool(name="stm", bufs=9))     # hid/inter/co TM fp16
            out32_p = ep(tc.tile_pool(name="out32", bufs=4))
            pt_p = ep(tc.tile_pool(name="pt", bufs=4))        # exp(scores) rings
            gel_p = ep(tc.tile_pool(name="gel", bufs=3))
            wsl_p = ep(tc.tile_pool(name="wsl", bufs=1))      # weight slabs
            row_p = ep(tc.tile_pool(name="row", bufs=4))      # [1,<=520] rows
            gb_p = ep(tc.tile_pool(name="gb", bufs=1))        # LN G/B broadcast fp32
            sc_p = ep(tc.tile_pool(name="sc", bufs=2))        # fp32 scratch
            s1_p = ep(tc.tile_pool(name="s1", bufs=1))        # [<=4,512] rows
            st_p = ep(tc.tile_pool(name="st", bufs=8))        # small stats
            ps_p = ep(tc.tile_pool(name="ps", bufs=6, space="PSUM"))
            dram_p = ep(tc.tile_pool(name="dram", bufs=2, space="DRAM"))

            identt = const_p.tile([128, 128], FP16, tag="ident", name="ident")
            nc.sync.dma_start(identt[:], ident_d[:])
            onest = const_p.tile([1, 128], FP16, tag="ones", name="ones")
            nc.sync.dma_start(onest[:], ones_d[:])
            knowfm = []
            for e in range(ET):
                t = know_p.tile([128, SK], FP16, tag="know", name="know")
                nc.sync.dma_start(t[:], know_fm_d[e * 128:(e + 1) * 128, :])
                knowfm.append(t)
            ownfm = xfm_p.tile([128, ET * CH], FP16, tag="xfm", name="ownfm")
            nc.sync.dma_start(ownfm[:], own_fm0[:])
            hid = []
            for t in range(TT):
                h = stm_p.tile([128, E], FP16, tag="stm", name="hid")
                nc.sync.dma_start(h[:], own_tm0[t * 128:(t + 1) * 128, :])
                hid.append(h)

            # ---------------- weight loaders (single-buffered slabs) ----
            def slab(dram, l, cols, tg, bufs=1):
                t = wsl_p.tile([128, cols], FP16, tag=tg, name=tg, bufs=bufs)
                nc.sync.dma_start(t[:], dram[l])
                return t

            def srow(dram, l, cols, tg="row"):
                t = row_p.tile([1, cols], FP16, tag=tg, name=tg, bufs=8)
                nc.sync.dma_start(t[:], dram[l])
                return t

            def sbias(dram, l, cols, tg="bias"):
                t = st_p.tile([128, cols], FP32, tag=tg, name=tg, bufs=8)
                nc.sync.dma_start(t[:], dram[l])
                return t

            def load_sa_kvq(l):
                return dict(
                    wq=slab(wq_sa_d, l, ET * ET * 128, "wq_sa"),
                    wk=slab(wk_sa_d, l, ET * ET * 128, "wk_sa"),
                    wv=slab(wv_sa_d, l, ET * HHW, "wv_sa"),
                    bq=sbias(bq_sa_d, l, ET), bk=sbias(bk_sa_d, l, ET),
                    rbv=srow(rbv_sa_d, l, HHW))

            def load_sa_o(l):
                return dict(wo=slab(wo_sa_d, l, ET * E, "wo_sa"),
                            rbo=srow(rbo_sa_d, l, E))

            def load_ca_kv(l):
                return dict(
                    wk=slab(wk_ca_d, l, ET * ET * 128, "wk_ca"),
                    wv=slab(wv_ca_d, l, ET * HHW, "wv_ca"),
                    bk=sbias(bk_ca_d, l, ET), rbv=srow(rbv_ca_d, l, HHW))

            def load_ca_qo(l):
                return dict(
                    wq=slab(wq_ca_d, l, ET * ET * 128, "wq_ca"),
                    wo=slab(wo_ca_d, l, ET * E, "wo_ca"),
                    bq=sbias(bq_ca_d, l, ET), rbo=srow(rbo_ca_d, l, E))

            def load_ffn_w(l):
                return dict(
                    w1=[slab(w1_d[l], q, 4 * ET * 128, "w1q", bufs=2)
                        for q in range(4)],
                    w2=[slab(w2_d[l], q, 4 * E, "w2q", bufs=2) for q in range(4)],
                    b1=sbias(b1_d, l, FT), rb2=srow(rb2_d, l, E))

            def load_ln(l):
                lr = s1_p.tile([1, E], FP32, tag="lnrow", name="lnrow")
                nc.sync.dma_start(lr[:], lng_d[l])
                G = gb_p.tile([128, E], FP32, tag="G", name="G")
                nc.gpsimd.partition_broadcast(G[:], lr[:])
                lr2 = s1_p.tile([1, E], FP32, tag="B", name="lnrow2")
                nc.sync.dma_start(lr2[:], lnb_d[l])
                Bt = gb_p.tile([128, E], FP32, tag="Bb", name="Bb")
                nc.gpsimd.partition_broadcast(Bt[:], lr2[:])
                return G, Bt

            # ---------------- compute helpers ----------------
            I32 = mybir.dt.int32

            def rsqrt4(mv8):
                """inv4 = rsqrt(var*E/(E-1)), minv4 = mean*inv4; vars at odd cols."""
                v4 = st_p.tile([128, 4], FP32, tag="v4", name="v4")
                nc.vector.tensor_scalar_mul(v4[:], mv8[:, 1:8:2], float(E) / (E - 1))
                h4 = st_p.tile([128, 4], FP32, tag="h4", name="h4")
                nc.vector.tensor_scalar_mul(h4[:], v4[:], 0.5)
                t1 = st_p.tile([128, 4], I32, tag="t1", name="t1")
                nc.vector.tensor_scalar(t1[:], in0=v4[:].bitcast(I32), scalar1=1,
                                        scalar2=None, op0=OP.arith_shift_right)
                y4 = st_p.tile([128, 4], FP32, tag="y4", name="y4")
                nc.vector.tensor_scalar(y4[:].bitcast(I32), in0=t1[:], scalar1=-1,
                                        scalar2=0x5f3759df, op0=OP.mult, op1=OP.add)
                for _ in range(2):
                    sq = st_p.tile([128, 4], FP32, tag="sq", name="sq")
                    nc.vector.tensor_mul(sq[:], y4[:], y4[:])
                    nc.vector.tensor_mul(sq[:], sq[:], h4[:])
                    nc.vector.tensor_scalar(sq[:], in0=sq[:], scalar1=-1.0,
                                            scalar2=1.5, op0=OP.mult, op1=OP.add)
                    nc.vector.tensor_mul(y4[:], y4[:], sq[:])
                m4 = st_p.tile([128, 4], FP32, tag="m4", name="m4")
                nc.vector.tensor_mul(m4[:], mv8[:, 0:8:2], y4[:])
                return y4, m4

            def ln_norm(xres, G, Bt, out):
                """out = G*(xres-mean)/(sqrt(bessel_var)+eps) + Bt."""
                stt = st_p.tile([128, 6], FP32, tag="bnst", name="bnst")
                nc.vector.bn_stats(out=stt[:], in_=xres[:])
                mv = st_p.tile([128, 2], FP32, tag="bnmv", name="bnmv")
                nc.vector.bn_aggr(out=mv[:], in_=stt[:])
                sd = st_p.tile([128, 1], FP32, tag="sd", name="sd")
                nc.scalar.activation(sd[:], mv[:, 1:2], AF.Sqrt, scale=float(E) / (E - 1))
                nc.vector.tensor_scalar_add(sd[:], sd[:], 1e-6)
                inv = st_p.tile([128, 1], FP32, tag="inv", name="inv")
                nc.vector.reciprocal_approx_fast(inv[:], sd[:])
                minv = st_p.tile([128, 1], FP32, tag="minv", name="minv")
                nc.vector.tensor_mul(minv[:], mv[:, 0:1], inv[:])
                tmp = sc_p.tile([128, E], FP32, tag="lntmp", name="lntmp")
                nc.vector.tensor_scalar(tmp[:], in0=xres[:], scalar1=inv[:],
                                        scalar2=minv[:], op0=OP.mult, op1=OP.subtract)
                nc.vector.tensor_mul(tmp[:], tmp[:], G[:])
                nc.vector.tensor_add(out[:], tmp[:], Bt[:])

            def transpose_to(dst, src_tile, t):
                """src [128tok, E] TM tile t -> dst[:, e*CH + t*128 ...] (fm)."""
                for e in range(ET):
                    tp = ps_p.tile([128, 128], FP16, tag="ps", name="ps")
                    nc.tensor.transpose(tp[:], src_tile[:, e * 128:(e + 1) * 128],
                                        identt[:])
                    nc.vector.tensor_copy(dst[:, e * CH + t * 128:e * CH + (t + 1) * 128],
                                          tp[:])

            def q_proj(src_fm, wq, bq):
                qs = []
                for e in range(ET):
                    pst = ps_p.tile([128, CH], FP32, tag="ps", name="ps")
                    for ei in range(ET):
                        nc.tensor.matmul(
                            pst[:], wq[:, (ei * ET + e) * 128:(ei * ET + e + 1) * 128],
                            src_fm[:, ei * CH:(ei + 1) * CH],
                            start=(ei == 0), stop=(ei == ET - 1))
                    qt = qfm_p.tile([128, CH], FP16, tag="qfm", name="qfm")
                    nc.vector.tensor_scalar_add(qt[:], pst[:], bq[:, e:e + 1])
                    qs.append(qt)
                return qs

            def kv_alloc():
                kfm = [kfm_p.tile([128, S], FP16, tag="kfm", name="kfm")
                       for _ in range(ET)]
                vsa = [vsa_p.tile([128, 4 * HHW], FP16, tag="vsa", name="vsa")
                       for _ in range(4)]
                return kfm, vsa

            def kv_chunk(w, hch, kfm, vsa, ch):
                """SA K/V for one chunk of hidden fm tiles into slot ch."""
                if True:
                    for e in range(ET):
                        pst = ps_p.tile([128, CH], FP32, tag="ps", name="ps")
                        for ei in range(ET):
                            nc.tensor.matmul(
                                pst[:],
                                w["wk"][:, (ei * ET + e) * 128:(ei * ET + e + 1) * 128],
                                hch[ei][:], start=(ei == 0), stop=(ei == ET - 1))
                        nc.scalar.activation(
                            kfm[e][:, ch * CH:(ch + 1) * CH], pst[:], AF.Identity,
                            bias=w["bk"][:, e:e + 1])
                    for lt in range(TT):
                        for half in range(2):
                            cs = half * (HHW // 2)
                            pst = ps_p.tile([128, HHW // 2], FP32, tag="ps", name="ps")
                            for ei in range(ET):
                                nc.tensor.matmul(
                                    pst[:], hch[ei][:, lt * 128:(lt + 1) * 128],
                                    w["wv"][:, ei * HHW + cs:ei * HHW + cs + HHW // 2],
                                    start=(ei == 0), stop=False)
                            nc.tensor.matmul(pst[:], onest[:],
                                             w["rbv"][:, cs:cs + HHW // 2],
                                             start=False, stop=True)
                            nc.scalar.activation(
                                vsa[ch][:, lt * HHW + cs:lt * HHW + cs + HHW // 2],
                                pst[:], AF.Copy)

            def make_ca_k(w):
                kca = [kca_p.tile([128, SK], FP16, tag="kca", name="kca")
                       for _ in range(ET)]
                for e in range(ET):
                    for cc in range(2):
                        pst = ps_p.tile([128, CH], FP32, tag="ps", name="ps")
                        for ei in range(ET):
                            nc.tensor.matmul(
                                pst[:],
                                w["wk"][:, (ei * ET + e) * 128:(ei * ET + e + 1) * 128],
                                knowfm[ei][:, cc * CH:(cc + 1) * CH],
                                start=(ei == 0), stop=(ei == ET - 1))
                        nc.scalar.activation(
                            kca[e][:, cc * CH:(cc + 1) * CH], pst[:], AF.Identity,
                            bias=w["bk"][:, e:e + 1])
                return kca

            def make_ca_v(w):
                vca = [vca_p.tile([128, 2 * HHW], FP16, tag="vca", name="vca")
                       for _ in range(4)]
                for kt in range(KT_CA):
                    for half in range(2):
                        cs = half * (HHW // 2)
                        pst = ps_p.tile([128, HHW // 2], FP32, tag="ps", name="ps")
                        for ei in range(ET):
                            nc.tensor.matmul(
                                pst[:], knowfm[ei][:, kt * 128:(kt + 1) * 128],
                                w["wv"][:, ei * HHW + cs:ei * HHW + cs + HHW // 2],
                                start=(ei == 0), stop=False)
                        nc.tensor.matmul(pst[:], onest[:], w["rbv"][:, cs:cs + HHW // 2],
                                         start=False, stop=True)
                        nc.scalar.activation(
                            vca[kt // 2][:, (kt % 2) * HHW + cs:(kt % 2) * HHW + cs + HHW // 2],
                            pst[:], AF.Copy)
                return vca

            def attention(qfm, kfm, vp_at, nkt, attn_tiles, sexp=False):
                PD = 2  # exp pipeline distance in kt tiles
                for hp in range(ET):
                    attps = [ps_p.tile([HW, CH], FP32, tag="attps", name="attps",
                                       bufs=2)
                             for _ in range(2)]
                    pts = {}

                    def scores(kt):
                        for j in (0, 1):
                            spt = ps_p.tile([128, CH], FP32, tag="ps", name="spt")
                            nc.tensor.matmul(
                                spt[:],
                                kfm[hp][j * 64:(j + 1) * 64, kt * 128:(kt + 1) * 128],
                                qfm[hp][j * 64:(j + 1) * 64, :], start=True, stop=True)
                            if j == 0 or not sexp:
                                pt = pt_p.tile([128, CH], FP16, tag="pte", name="pte")
                                nc.scalar.activation(pt[:], spt[:], AF.Exp, scale=0.125)
                                pts[kt, j] = pt[:]
                            else:
                                pti = pt_p.tile([128, CH], FP16, tag="ptv", name="ptv")
                                nc.vector.tensor_scalar(pti[:].bitcast(I16), in0=spt[:],
                                                        scalar1=SEXP_A, scalar2=SEXP_B,
                                                        op0=OP.mult, op1=OP.add)
                                pts[kt, j] = pti[:]

                    def pv(kt):
                        for j in (0, 1):
                            h = hp * 2 + j
                            vtile, col0 = vp_at(kt, h)
                            nc.tensor.matmul(attps[j][:], vtile[:, col0:col0 + HW],
                                             pts.pop((kt, j)),
                                             start=(kt == 0), stop=(kt == nkt - 1))

                    for kt in range(nkt):
                        scores(kt)
                        if kt >= PD:
                            pv(kt - PD)
                    for kt in range(nkt - PD, nkt):
                        pv(kt)
                    for j in (0, 1):
                        den = s1_p.tile([1, CH], FP32, tag="den", name="den")
                        nc.scalar.activation(den[:], attps[j][64:65, :], AF.Copy)
                        rec = s1_p.tile([1, CH], FP32, tag="rec", name="rec")
                        nc.vector.reciprocal_approx_fast(rec[:], den[:])
                        rb = sc_p.tile([64, CH], FP32, tag="rb", name="rb")
                        nc.gpsimd.partition_broadcast(rb[:], rec[:])
                        nc.vector.tensor_mul(attn_tiles[hp][j * 64:(j + 1) * 64, :],
                                             attps[j][0:64, :], rb[:])

            def ln_finish(xs, mv8, G, Bt, out_tiles):
                inv4, minv4 = rsqrt4(mv8)
                for t in range(TT):
                    tmp = sc_p.tile([128, E], FP32, tag="lntmp", name="lntmp")
                    nc.vector.tensor_scalar(tmp[:], in0=xs[t][:],
                                            scalar1=inv4[:, t:t + 1],
                                            scalar2=minv4[:, t:t + 1],
                                            op0=OP.mult, op1=OP.subtract)
                    nc.vector.tensor_mul(tmp[:], tmp[:], G[:])
                    nc.vector.tensor_add(out_tiles[t][:], tmp[:], Bt[:])

            def out_proj_ln(attn_tiles, w, res, G, Bt, out_tiles):
                mv8 = st_p.tile([128, 8], FP32, tag="mv8", name="mv8", bufs=4)
                xs = []
                for t in range(TT):
                    pst = ps_p.tile([128, E], FP32, tag="ps", name="ps")
                    for ei in range(ET):
                        nc.tensor.matmul(pst[:], attn_tiles[ei][:, t * 128:(t + 1) * 128],
                                         w["wo"][:, ei * E:(ei + 1) * E],
                                         start=(ei == 0), stop=False)
                    nc.tensor.matmul(pst[:], onest[:], w["rbo"][:], start=False, stop=True)
                    xres = sc_p.tile([128, E], FP32, tag="xres", name="xres", bufs=4)
                    nc.vector.tensor_add(xres[:], pst[:], res[t][:])
                    stt = st_p.tile([128, 6], FP32, tag="bnst", name="bnst")
                    nc.vector.bn_stats(out=stt[:], in_=xres[:])
                    nc.vector.bn_aggr(out=mv8[:, 2 * t:2 * t + 2], in_=stt[:])
                    xs.append(xres)
                ln_finish(xs, mv8, G, Bt, out_tiles)

            def hch_from_sen(j):
                """Remote chunk (pid+j)%4 of sen, dynamic column offset."""
                pid = nc.sync.partition_id()
                sel = (pid + j) % 4
                tiles = []
                for ei in range(ET):
                    t = hch_p.tile([128, CH], FP16, tag="hch", name="hch")
                    nc.sync.dma_start(t[:], sen_fm[ei * 128:(ei + 1) * 128,
                                                   ts(sel, CH)])
                    tiles.append(t)
                return tiles

            def hch_from_ag(ag_out, j):
                """Remote chunk (pid+j)%4 of the gathered hidden, dynamic rows."""
                pid = nc.sync.partition_id()
                sel = (pid + j) % 4
                tiles = []
                for ei in range(ET):
                    t = hch_p.tile([128, CH], FP16, tag="hch", name="hch")
                    nc.sync.dma_start(t[:], ag_out[ts(sel * ET + ei, 128), :])
                    tiles.append(t)
                return tiles

            # ---------------- bootstrap: layer-0 K/V + CA K/V ----------------
            nc.cache_partition_id()
            sa_kvq = load_sa_kvq(0)
            ca_kv = load_ca_kv(0)
            G, Bt = load_ln(0)
            kfm, vsa = kv_alloc()
            own_hch = [ownfm[:, ei * CH:(ei + 1) * CH] for ei in range(ET)]
            kv_chunk(sa_kvq, own_hch, kfm, vsa, 0)
            for j in (1, 2, 3):
                kv_chunk(sa_kvq, [t[:] for t in hch_from_sen(j)], kfm, vsa, j)
            qsa = q_proj(ownfm, sa_kvq["wq"], sa_kvq["bq"])
            kca = make_ca_k(ca_kv)
            vca = make_ca_v(ca_kv)
            ag_out_cur = None
            kfm_next = vsa_next = None

            for l in range(L):
                with nc.named_scope(f"L{l}"):
                    if l > 0:
                        kfm, vsa = kfm_next, vsa_next
                        for j in (1, 2, 3):
                            kv_chunk(sa_kvq, [t[:] for t in hch_from_ag(ag_out_cur, j)],
                                     kfm, vsa, j)
                    sa_o = load_sa_o(l)
                    ca_qo = load_ca_qo(l)
                    ffn_w = load_ffn_w(l)
                    if l < L - 1:
                        ca_kv_next = load_ca_kv(l + 1)

                    # ---- SA attention ----
                    attn = [attn_p.tile([128, CH], FP16, tag="attn", name="attn")
                            for _ in range(ET)]
                    with nc.named_scope("sa"):
                        attention(qsa, kfm,
                                  lambda kt, h: (vsa[kt // 4], (kt % 4) * HHW + h * HW),
                                  KT_SA, attn, sexp=SEXP_MODE in ("1", "sa"))

                    inter = [stm_p.tile([128, E], FP16, tag="stm", name="inter")
                             for _ in range(TT)]
                    with nc.named_scope("oln1"):
                        out_proj_ln(attn, sa_o, hid, G, Bt, inter)
                        interfm = xfm_p.tile([128, ET * CH], FP16, tag="xfm",
                                             name="interfm")
                        for t in range(TT):
                            transpose_to(interfm, inter[t], t)

                    # ---- CA ----
                    with nc.named_scope("ca"):
                        qca = q_proj(interfm, ca_qo["wq"], ca_qo["bq"])
                        if l < L - 1:
                            sa_kvq_next = load_sa_kvq(l + 1)
                        attn2 = [attn_p.tile([128, CH], FP16, tag="attn", name="attn2")
                                 for _ in range(ET)]
                        attention(qca, kca,
                                  lambda kt, h: (vca[kt // 2], (kt % 2) * HHW + h * HW),
                                  KT_CA, attn2, sexp=SEXP_MODE in ("1", "ca"))
                    co = [stm_p.tile([128, E], FP16, tag="stm", name="co")
                          for _ in range(TT)]
                    with nc.named_scope("oln2"):
                        out_proj_ln(attn2, ca_qo, inter, G, Bt, co)
                        if l < L - 1:
                            kca = make_ca_k(ca_kv_next)
                        cofm = xfm_p.tile([128, ET * CH], FP16, tag="xfm", name="cofm")
                        for t in range(TT):
                            transpose_to(cofm, co[t], t)

                    # ---- FFN (h1/gelu/h2 interleaved, distance 2) ----
                    with nc.named_scope("ffn"):
                        if l == L - 1:
                            hidn = [out32_p.tile([128, E], FP32, tag="out32",
                                                 name="out32") for _ in range(TT)]
                        else:
                            hidn = [stm_p.tile([128, E], FP16, tag="stm", name="hidn")
                                    for _ in range(TT)]
                        h2ps = [ps_p.tile([128, E], FP32, tag="ps", name="ps")
                                for _ in range(TT)]
                        gel = {}

                        def h2_emit(ft):
                            gt = gel.pop(ft)
                            for t in range(TT):
                                nc.tensor.matmul(h2ps[t][:], gt[:, t * 128:(t + 1) * 128],
                                                 ffn_w["w2"][ft // 4][:,
                                                 (ft % 4) * E:(ft % 4 + 1) * E],
                                                 start=(ft == 0), stop=False)

                        for ft in range(FT):
                            pst = ps_p.tile([128, CH], FP32, tag="ps", name="ps")
                            w1q = ffn_w["w1"][ft // 4]
                            for ei in range(ET):
                                nc.tensor.matmul(
                                    pst[:],
                                    w1q[:, ((ft % 4) * ET + ei) * 128:
                                        ((ft % 4) * ET + ei + 1) * 128],
                                    cofm[:, ei * CH:(ei + 1) * CH],
                                    start=(ei == 0), stop=(ei == ET - 1))
                            gt = gel_p.tile([128, CH], FP16, tag="gel", name="gel")
                            nc.scalar.activation(gt[:], pst[:], AF.Gelu,
                                                 bias=ffn_w["b1"][:, ft:ft + 1])
                            gel[ft] = gt
                            if ft >= 2:
                                h2_emit(ft - 2)
                        h2_emit(FT - 2)
                        h2_emit(FT - 1)
                        mv8 = st_p.tile([128, 8], FP32, tag="mv8", name="mv8",
                                        bufs=4)
                        xs = []
                        for t in range(TT):
                            nc.tensor.matmul(h2ps[t][:], onest[:], ffn_w["rb2"][:],
                                             start=False, stop=True)
                            xres = sc_p.tile([128, E], FP32, tag="xres", name="xres",
                                             bufs=4)
                            nc.vector.tensor_add(xres[:], h2ps[t][:], co[t][:])
                            stt = st_p.tile([128, 6], FP32, tag="bnst", name="bnst")
                            nc.vector.bn_stats(out=stt[:], in_=xres[:])
                            nc.vector.bn_aggr(out=mv8[:, 2 * t:2 * t + 2], in_=stt[:])
                            xs.append(xres)
                        ln_finish(xs, mv8, G, Bt, hidn)
                        if l == L - 1:
                            for t in range(TT):
                                nc.sync.dma_start(out_d[t * 128:(t + 1) * 128, :],
                                                  hidn[t][:])

                    # ---- boundary: AllGather hidden; CA K/V + next Q fill it ----
                    if l < L - 1:
                        with nc.named_scope("bnd"):
                            ownfm_n = xfm_p.tile([128, ET * CH], FP16, tag="xfm",
                                                 name="ownfm_n")
                            for t in range(TT):
                                transpose_to(ownfm_n, hidn[t], t)
                            ag_in = dram_p.tile([E, CH], FP16, tag="agin", name="agin")
                            for e in range(ET):
                                nc.scalar.dma_start(
                                    ag_in[e * 128:(e + 1) * 128, :],
                                    ownfm_n[:, e * CH:(e + 1) * CH])
                            ag_out_cur = dram_p.tile([4 * E, CH], FP16, tag="agout",
                                                     name="agout")
                            nc.gpsimd.collective_compute(
                                "AllGather", OP.bypass, replica_groups=GROUPS,
                                ins=[ag_in.opt()], outs=[ag_out_cur.opt()])
                            kfm_next, vsa_next = kv_alloc()
                            own_h = [ownfm_n[:, ei * CH:(ei + 1) * CH]
                                     for ei in range(ET)]
                            kv_chunk(sa_kvq_next, own_h, kfm_next, vsa_next, 0)
                            qsa = q_proj(ownfm_n, sa_kvq_next["wq"], sa_kvq_next["bq"])
                            vca = make_ca_v(ca_kv_next)
                            Gn, Btn = load_ln(l + 1)
                        sa_kvq, ca_kv, G, Bt = sa_kvq_next, ca_kv_next, Gn, Btn
                        hid = hidn

    nc.compile()
    return nc


def _pack_ee(w):
    """[L,E,E] -> [L,128, ET*ET*128] slab: cols (ei,e,c), lhsT tile (ei,e)."""
    return np.ascontiguousarray(
        w.reshape(L, ET, 128, ET, 128).transpose(0, 2, 1, 3, 4)
        .reshape(L, 128, ET * ET * 128).astype(np.float16))


def _fm_pack(x_fm):
    """[E, T] -> [128, ET*T] (cols (e,t))."""
    t = x_fm.shape[1]
    return np.ascontiguousarray(
        x_fm.reshape(ET, 128, t).transpose(1, 0, 2).reshape(128, ET * t)
        .astype(np.float16))


def _prep_inputs(sen, know, sa_qkv_w, sa_qkv_b, sa_out_w, sa_out_b,
                 ca_qkv_w, ca_qkv_b, ca_out_w, ca_out_b,
                 ff_w1, ff_b1, ff_w2, ff_b2, ln_g, ln_b):
    f16, f32 = np.float16, np.float32

    def padv(w, b):  # [L,E,E],[L,E] -> [L,128,ET*HHW], [L,1,HHW]
        wp = np.zeros((L, E, H, HW), f32)
        wp[:, :, :, :D] = w.reshape(L, E, H, D)
        bp = np.zeros((L, H, HW), f32)
        bp[:, :, :D] = b.reshape(L, H, D)
        bp[:, :, D] = 1.0
        wsl = wp.reshape(L, ET, 128, H * HW).transpose(0, 2, 1, 3).reshape(
            L, 128, ET * HHW)
        return (np.ascontiguousarray(wsl.astype(f16)),
                np.ascontiguousarray(bp.reshape(L, 1, HHW).astype(f16)))

    wv_sa_p, rbv_sa_h = padv(sa_qkv_w[:, 2], sa_qkv_b[:, 2])
    wv_ca_p, rbv_ca_h = padv(ca_qkv_w[:, 2], ca_qkv_b[:, 2])

    def pack_o(w):  # [L,E,E] -> [L,128,ET*E]
        return np.ascontiguousarray(
            w.reshape(L, ET, 128, E).transpose(0, 2, 1, 3).reshape(L, 128, ET * E)
            .astype(f16))

    # w1: [L,E,F] -> quarter-slabs [L,4,128,4*ET*128], cols (ft%4, ei, c)
    w1q = (ff_w1.reshape(L, ET, 128, 4, 4, 128)      # (ei,p,q,ftq,c)
           .transpose(0, 3, 2, 4, 1, 5)              # (L,q,p,ftq,ei,c)
           .reshape(L, 4, 128, 4 * ET * 128))
    # w2: [L,F,E] -> quarter-slabs [L,4,128,4*E], cols (ft%4, c)
    w2q = (ff_w2.reshape(L, 4, 4, 128, E)            # (q,ftq,p,c)
           .transpose(0, 1, 3, 2, 4)                 # (L,q,p,ftq,c)
           .reshape(L, 4, 128, 4 * E))

    common = {
        "ident": np.eye(128, dtype=f16),
        "ones": np.ones((1, 128), f16),
        "wq_sa": _pack_ee(sa_qkv_w[:, 0]), "wk_sa": _pack_ee(sa_qkv_w[:, 1]),
        "wv_sa": wv_sa_p, "wo_sa": pack_o(sa_out_w),
        "wq_ca": _pack_ee(ca_qkv_w[:, 0]), "wk_ca": _pack_ee(ca_qkv_w[:, 1]),
        "wv_ca": wv_ca_p, "wo_ca": pack_o(ca_out_w),
        "w1": np.ascontiguousarray(w1q.astype(f16)),
        "w2": np.ascontiguousarray(w2q.astype(f16)),
        "bq_sa": np.ascontiguousarray(
            sa_qkv_b[:, 0].reshape(L, ET, 128).transpose(0, 2, 1)),
        "bk_sa": np.ascontiguousarray(
            sa_qkv_b[:, 1].reshape(L, ET, 128).transpose(0, 2, 1)),
        "bq_ca": np.ascontiguousarray(
            ca_qkv_b[:, 0].reshape(L, ET, 128).transpose(0, 2, 1)),
        "bk_ca": np.ascontiguousarray(
            ca_qkv_b[:, 1].reshape(L, ET, 128).transpose(0, 2, 1)),
        "b1": np.ascontiguousarray(ff_b1.reshape(L, FT, 128).transpose(0, 2, 1)),
        "rbv_sa": rbv_sa_h, "rbv_ca": rbv_ca_h,
        "rbo_sa": np.ascontiguousarray(sa_out_b[:, None, :].astype(f16)),
        "rbo_ca": np.ascontiguousarray(ca_out_b[:, None, :].astype(f16)),
        "rb2": np.ascontiguousarray(ff_b2[:, None, :].astype(f16)),
        "lng": np.ascontiguousarray(ln_g[:, None, :]),
        "lnb": np.ascontiguousarray(ln_b[:, None, :]),
    }
    in_maps = []
    for core in range(NCORES):
        g, c = core // 4, core % 4
        m = dict(common)
        m["sen_fm"] = np.ascontiguousarray(sen[g].T.astype(f16))
        m["own_fm0"] = _fm_pack(sen[g, c * CH:(c + 1) * CH].T)
        m["own_tm0"] = np.ascontiguousarray(sen[g, c * CH:(c + 1) * CH].astype(f16))
        m["know_fm"] = np.ascontiguousarray(know[g].T.astype(f16))
        in_maps.append(m)
    return in_maps


def kernel(**inputs):
    inputs = {k: np.asarray(v, dtype=np.float32) for k, v in inputs.items()}
    if "nc" not in _CACHE:
        _CACHE["nc"] = _build()
    nc = _CACHE["nc"]
    in_maps = _prep_inputs(**inputs)
    res = run_bass_kernel_spmd(nc, in_maps, list(range(NCORES)))
    out = np.empty((B, S, E), np.float32)
    for core in range(NCORES):
        g, c = core // 4, core % 4
        out[g, c * CH:(c + 1) * CH] = res.results[core]["out_tm"]
    return out
